# revision 43
# baseline (speedup 1.0000x reference)
"""Trainium2 Bass kernel for the blob-layer problem.

Computes out[b, c] = sum_hw x[b, hw] * curves[hw, c] / (H*W) where
curves[hw, c] = clip(factor_c * exp(-((xs-px_c)^2 + (ys-py_c)^2)/s2_c) * w_c).

Strategy (8 NeuronCores, SPMD, one shared program):
- Shard the pixel (contraction) dim into 8 y-bands of 28 rows; each core
  computes a partial (B, NC) output in slot space which the host scatters.
- Band pixels are laid out x-major (column-major image order), so each
  128-pixel tile spans only ~4.6 of the 224 x-columns. Curves are sorted
  by px and assigned to NC slots following the global px-quantile, so the
  set of curves within reach of a tile is a short contiguous slot window
  that is aligned across cores (the program, and hence the static window
  bounds, is shared by all 8 cores; per-tile windows are the union of the
  per-core windows).
- Pruning is amplitude-aware: curve c is kept for margin m when
  m^2/s2_c <= T + ln(|fw_c|/max|fw|) (T_KEEP for the y-band criterion,
  T_WIN for the per-tile x window); dropped contributions are bounded by
  e^-T * max|fw| / npix per pixel.
- grid is a rank-4 bilinear form: a K=12 stacked fp16 hi/lo matmul
  (rows [Bh;Bh;Bl] x [Ah;Al;Ah], ~2^-22 precision at full PE rate)
  produces M = -grid/s2 + ln(|fw_c|/npix) for a 128-pixel tile against
  its slot window directly in PSUM. ScalarE Exp gives e = curves_c/npix
  (up to sign) in bf16, and bf16 matmuls accumulate x^T e into two
  128-batch PSUM banks. The per-curve sign is applied host-side during
  the gather, so no on-device rescale pass is needed.
- PSUM output banks are zero-filled once up front and every main matmul
  accumulates (start=False): a slot's first touching tile varies per
  slot, so start-flag zeroing can't be used.
- The clip never binds when max|factor*w| <= CAP (e <= 1), which holds
  for these inputs (asserted host-side).
"""
import os
import sys

sys.path.insert(0, "/opt/trn_rl_repo")

import ml_dtypes
import numpy as np

import concourse.bass as bass
import concourse.bacc as bacc
import concourse.tile as tile
from concourse import mybir
from concourse.bass_utils import run_bass_kernel_spmd

H, W, B, C = 224, 224, 256, 1024
NDEV = 8
ROWS = H // NDEV          # 28 rows per band
HWD = ROWS * W            # 6272 pixels per band
NT = HWD // 128           # 49 pixel tiles per band
EPS = 0.001
CAP = 2000.0
NPIX = float(H * W)
NC = 480                  # slot count (fits one 2KB fp32 PSUM bank)
T_KEEP = 7.0              # y-band keep threshold (amplitude-adjusted)
T_WIN = 5.0               # per-tile x-window threshold (amplitude-adjusted)
T_FLOOR = 1.0

last_results = None       # BassKernelResults of the most recent run (for profiling)


def _build_program(windows, skew=3, grp_t=8):
    """Emit the SPMD Bass program. `windows` is a list of NT (lo, hi) slot
    ranges (shared across cores). Tiles are processed in groups of grp_t:
    each group's G matmuls pack their outputs contiguously into a 2-bank
    PSUM tile so a single ScalarE Exp covers the whole group (the fixed
    cost per ACTIVATE dominates at these window widths)."""
    nc = bacc.Bacc()
    f32 = mybir.dt.float32
    f16 = mybir.dt.float16
    bf16 = mybir.dt.bfloat16

    # partition-major x layout: each partition's bytes are contiguous per
    # chunk, so the load DMAs run long linear descriptors at full rate
    d_xT = nc.declare_dram_parameter("xT", [128, NT, B], bf16, isOutput=False)
    d_Wp = nc.declare_dram_parameter("Wp", [12, HWD], f16, isOutput=False)
    d_Mv = nc.declare_dram_parameter("Mv", [12, NC], f16, isOutput=False)
    d_out = nc.declare_dram_parameter("out", [2, 128, NC], f32, isOutput=True)

    # groups of tiles whose packed G widths fit a 2-bank (1024-fp32) PSUM
    # tile; a single matmul's output slice must stay inside one 512-fp32
    # bank, so offsets that would straddle the boundary are bumped to 512
    groups = []
    cur, acc = [], 0
    for t in range(NT):
        lo, hi = windows[t]
        w = hi - lo
        if w == 0:
            continue
        off = acc
        if off < 512 and off + w > 512:
            off = 512
        if len(cur) == grp_t or off + w > 1024:
            groups.append(cur)
            cur, off = [], 0
        cur.append((t, lo, hi, off))
        acc = off + w
    if cur:
        groups.append(cur)

    with tile.TileContext(nc) as tc:
        with (
            tc.tile_pool(name="const", bufs=1) as cpool,
            tc.tile_pool(name="ep", bufs=5) as ep,
            tc.tile_pool(name="op", bufs=1) as op,
            tc.tile_pool(name="psG", bufs=3, space="PSUM") as psG,
            tc.tile_pool(name="psO", bufs=1, space="PSUM") as psO,
        ):
            Wp = cpool.tile([12, HWD], f16, tag="Wp")
            Mv = cpool.tile([12, NC], f16, tag="Mv")
            # the pieces gating the first G matmuls ride first on separate
            # queues: Wp's head on sync, Mv (tiny) on scalar
            nc.sync.dma_start(Wp[:, 0 : 25 * 128], d_Wp[:, 0 : 25 * 128])
            nc.scalar.dma_start(Mv[:], d_Mv[:])
            nc.scalar.dma_start(Wp[:, 25 * 128 : 37 * 128], d_Wp[:, 25 * 128 : 37 * 128])
            nc.sync.dma_start(Wp[:, 37 * 128 :], d_Wp[:, 37 * 128 :])

            # whole x band stays SBUF-resident (25KB/partition). Early DMA
            # bandwidth is scarce (the path ramps over the first ~10us), so
            # only the first chunks are triggered up front; the rest are
            # triggered from inside the compute loop (the sync/gpsimd
            # engines are idle there), arriving just in time without
            # starving the Wp/Mv loads that gate compute.
            xfull = cpool.tile([128, NT * B], bf16, tag="xfull")

            def load_x(q, t0, t1):
                q.dma_start(
                    xfull[:, t0 * B : t1 * B].rearrange(
                        "p (t b) -> p t b", t=t1 - t0
                    ),
                    d_xT[:, t0:t1],
                )

            load_x(nc.gpsimd, 0, 3)
            load_x(nc.gpsimd, 3, 6)
            load_x(nc.gpsimd, 6, 9)
            deferred = [
                (nc.sync, 9, 15), (nc.gpsimd, 15, 21), (nc.sync, 21, 27),
                (nc.gpsimd, 27, 35), (nc.sync, 35, 42), (nc.gpsimd, 42, 49),
            ]

            Op0 = psO.tile([128, NC], f32, tag="op0")
            Op1 = psO.tile([128, NC], f32, tag="op1")
            nc.vector.memset(Op0[:], 0.0)
            nc.vector.memset(Op1[:], 0.0)

            def emit_main(group, e):
                for t, lo, hi, off in group:
                    last = t == NT - 1
                    w = hi - lo
                    for bb, Opx in ((0, Op0), (1, Op1)):
                        nc.tensor.matmul(
                            Opx[:, lo:hi],
                            xfull[:, t * B + bb * 128 : t * B + (bb + 1) * 128],
                            e[:, off : off + w],
                            start=False,
                            stop=last,
                            skip_group_check=True,
                        )

            # stream finalized output slots out while compute continues:
            # slot s takes its last accumulation from the last tile whose
            # window reaches it, so once every remaining window starts at
            # lo >= F, slots [0, F) are final and can be copied + DMAed.
            out_sb = op.tile([128, 2 * NC], f32, tag="out")
            emitted = [0]
            outq = [nc.sync, nc.gpsimd]

            def flush_out(F, piece):
                E = emitted[0]
                if F <= E:
                    return
                nc.vector.tensor_copy(out_sb[:, E:F], Op0[:, E:F])
                nc.vector.tensor_copy(
                    out_sb[:, NC + E : NC + F], Op1[:, E:F]
                )
                q = outq[piece % 2]
                q.dma_start(d_out[0][:, E:F], out_sb[:, E:F])
                q.dma_start(d_out[1][:, E:F], out_sb[:, NC + E : NC + F])
                emitted[0] = F

            pending = []
            npiece = [0]
            flushq = []  # (boundary F, min main-group index to emit it at)
            nmain = [0]

            def emit_main_and_flush(group, e):
                # emit any flush whose PE-side semaphore has had 2 groups
                # to propagate (a copy's completion sem takes ~1us to reach
                # the PE; without lag every flush stalls the pipeline)
                if flushq and nmain[0] >= flushq[0][1]:
                    F, _ = flushq.pop(0)
                    flush_out(F, npiece[0])
                    npiece[0] += 1
                emit_main(group, e)
                nmain[0] += 1
                t_last = group[-1][0]
                nxt = t_last + 1
                F = windows[nxt][0] if nxt < NT else NC
                pend_F = flushq[-1][0] if flushq else emitted[0]
                thr = 96 if t_last < 36 else 48
                if F - pend_F >= thr and nxt < NT:
                    flushq.append((F, nmain[0] + 1))

            for gi, group in enumerate(groups):
                if gi < len(deferred):
                    load_x(*deferred[gi])
                Gp = psG.tile([128, 1024], f32, tag="Gp")
                for t, lo, hi, off in group:
                    nc.tensor.matmul(
                        Gp[:, off : off + (hi - lo)],
                        Wp[:, t * 128 : (t + 1) * 128],
                        Mv[:, lo:hi],
                        start=True,
                        stop=True,
                        skip_group_check=True,
                    )
                wg = group[-1][3] + group[-1][2] - group[-1][1]
                e = ep.tile([128, 1024], bf16, tag="e")
                nc.scalar.activation(
                    e[:, 0:wg], Gp[:, 0:wg], mybir.ActivationFunctionType.Exp
                )
                pending.append((group, e))
                if len(pending) > skew:
                    emit_main_and_flush(*pending.pop(0))
            while pending:
                emit_main_and_flush(*pending.pop(0))
            for F, _ in flushq:
                flush_out(F, npiece[0])
                npiece[0] += 1
            flush_out(NC, npiece[0])

    nc.compile()
    return nc


def _prepare(x, positions, sigmas, curve_weights, xs, ys):
    x = np.asarray(x, dtype=np.float32)
    px = np.asarray(positions, dtype=np.float64)[0, 0, :, 1]
    py = np.asarray(positions, dtype=np.float64)[0, 0, :, 0]
    sg = np.asarray(sigmas, dtype=np.float64)[0, 0]
    w = np.asarray(curve_weights, dtype=np.float64)[0, 0]
    xs = np.asarray(xs, dtype=np.float64)
    ys = np.asarray(ys, dtype=np.float64)

    s2 = 2.0 * sg * sg + EPS
    factor = 1.0 / (2.0 * np.pi * sg * sg + EPS)
    fw = factor * w
    # clip(curves) is identity when max|factor*w| <= CAP since exp(...) <= 1
    assert np.abs(fw).max() <= CAP, "clip binds; folded-scale scheme invalid"

    absfw = np.maximum(np.abs(fw), 1e-12)
    lnr = np.log(absfw / absfw.max())
    Tk = np.maximum(T_KEEP + lnr, T_FLOOR)
    Tw = np.maximum(T_WIN + lnr, T_FLOOR)

    gorder = np.argsort(px, kind="stable")
    grank = np.empty(C, dtype=np.int64)
    grank[gorder] = np.arange(C)

    los = np.full((NDEV, NT), np.iinfo(np.int64).max, dtype=np.int64)
    his = np.zeros((NDEV, NT), dtype=np.int64)
    band = []
    for d in range(NDEV):
        h0 = d * ROWS
        y0, y1 = ys[h0, 0], ys[h0 + ROWS - 1, 0]
        ymarg = np.maximum(np.maximum(y0 - py, py - y1), 0.0)
        kept = np.where(ymarg * ymarg / s2 <= Tk)[0]
        order = kept[np.argsort(px[kept], kind="stable")]
        nk = len(order)
        assert nk <= NC, f"band {d} keeps {nk} > NC={NC} columns"
        # monotone slot assignment following the global px-quantile so the
        # per-tile windows line up across bands
        ideal = (grank[order] * NC) // C
        slot = np.zeros(nk, dtype=np.int64)
        s = -1
        for i in range(nk):
            s = max(s + 1, int(ideal[i]))
            slot[i] = s
        if nk and slot[-1] > NC - 1:
            slot[-1] = NC - 1
            for i in range(nk - 2, -1, -1):
                slot[i] = min(slot[i], slot[i + 1] - 1)
        pxs = px[order]
        ym = ymarg[order]
        for t in range(NT):
            xi0, xi1 = (t * 128) // ROWS, (t * 128 + 127) // ROWS
            xx0, xx1 = xs[0, xi0], xs[0, min(xi1, W - 1)]
            xmarg = np.maximum(np.maximum(xx0 - pxs, pxs - xx1), 0.0)
            act = np.where((ym * ym + xmarg * xmarg) / s2[order] <= Tw[order])[0]
            if len(act):
                los[d, t] = slot[act[0]]
                his[d, t] = slot[act[-1]] + 1
        band.append((order, slot))

    lo_u = los.min(axis=0)
    hi_u = his.max(axis=0)
    # enforce monotone windows (they already are, up to ties) so every slot
    # in [lo_0, hi_last) is covered by a contiguous run of tiles
    for t in range(1, NT):
        hi_u[t] = max(hi_u[t], hi_u[t - 1]) if hi_u[t] else hi_u[t - 1]
        lo_u[t] = max(min(lo_u[t], hi_u[t]), lo_u[t - 1])
    windows = [
        (int(min(lo_u[t], hi_u[t])), int(hi_u[t])) for t in range(NT)
    ]

    in_maps = []
    gathers = []
    for d in range(NDEV):
        h0 = d * ROWS
        rows = slice(h0, h0 + ROWS)
        # x-major pixel order: p = xi*ROWS + yi; ys is taken relative to
        # the band start (y0 folded into py), which keeps the fp16 hi/lo
        # magnitudes small
        y0 = ys[h0, 0]
        xs_b = xs[rows].T.ravel()
        ys_b = ys[rows].T.ravel() - y0
        Bm = np.stack(
            [xs_b, ys_b, np.ones(HWD), xs_b * xs_b + ys_b * ys_b]
        )

        order, slot = band[d]
        pyd = py[order] - y0
        lnf = np.log(np.abs(fw[order]) + 1e-300) - np.log(NPIX)
        Am = np.zeros((4, NC))
        Am[2, :] = -60.0
        Am[3, :] = -1.0
        Am[0, slot] = 2.0 * px[order] / s2[order]
        Am[1, slot] = 2.0 * pyd / s2[order]
        Am[2, slot] = -(px[order] ** 2 + pyd**2) / s2[order] + lnf
        Am[3, slot] = -1.0 / s2[order]
        Bh = Bm.astype(np.float16)
        Bl = (Bm - Bh.astype(np.float64)).astype(np.float16)
        Ah = Am.astype(np.float16)
        Al = (Am - Ah.astype(np.float64)).astype(np.float16)
        # K=12 stacked hi/lo split: [Bh;Bh;Bl]^T @ [Ah;Al;Ah]
        Wp = np.concatenate([Bh, Bh, Bl], axis=0)
        Mv = np.concatenate([Ah, Al, Ah], axis=0)

        xT = np.ascontiguousarray(
            x[:, rows, :].transpose(0, 2, 1).reshape(B, HWD).T
        ).reshape(NT, 128, B).transpose(1, 0, 2)
        xT = np.ascontiguousarray(xT).astype(ml_dtypes.bfloat16)

        in_maps.append({"xT": xT, "Wp": Wp, "Mv": Mv})
        gathers.append((order, slot, np.sign(fw[order]).astype(np.float32)))
    return windows, in_maps, gathers


def _gather(results, gathers):
    out = np.zeros((B, C), np.float32)
    for d in range(NDEV):
        order, slot, sgn = gathers[d]
        dev = np.asarray(results[d]["out"], np.float32).reshape(B, NC)
        out[:, order] += dev[:, slot] * sgn
    return out


def kernel(x, positions, sigmas, curve_weights, xs, ys):
    global last_results
    windows, in_maps, gathers = _prepare(
        x, positions, sigmas, curve_weights, xs, ys
    )
    nc = _build_program(windows)
    trace = bool(os.environ.get("BLOB_TRACE"))
    last_results = run_bass_kernel_spmd(
        nc, in_maps, list(range(NDEV)), trace=trace
    )
    return _gather(last_results.results, gathers)
